# revision 1
# baseline (speedup 1.0000x reference)
"""Trainium2 Bass kernel for AttnBlock (GroupNorm + 1x1-conv QKV + 4096x4096
attention + output projection + residual), B=4, C=512, H=W=64.

Sharding: 8 cores = 4 samples x 2 query-halves. Each core receives its
sample's x rolled so that "its" 2048 query columns are columns 0:2048 —
attention is invariant to key order, so one identical SPMD program serves
all 8 cores (no collectives, no per-core program specialization).

Per-core pipeline (all layouts [channel-on-partition, pixel-on-free] unless
noted):
  1. GroupNorm(32 groups): bn_stats per partition, cross-partition group
     combine via a tiny matmul with a group-selector matrix, normalize to
     h (bf16).
  2. q = qw@h (2048 cols), k = kw@h (4096 cols), vT = h^T@vw^T (v transposed
     so the attention O-matmul can contract over keys on the partition dim).
  3. For each 512-wide query chunk: S^T tiles = k^T q (keys on partitions),
     exp on the scalar engine (no max-subtraction needed: scores ~ N(0,1)),
     unnormalized O accumulated over all 32 key tiles, row-sums via a
     ones-vector matmul, then O * (1/rowsum) and the output projection with
     bias + residual.
"""

import sys

import numpy as np

try:
    import concourse.bass as bass
except ImportError:  # harness environments differ in sys.path
    sys.path.insert(0, "/opt/trn_rl_repo")
    import concourse.bass as bass

from contextlib import ExitStack

import ml_dtypes

import concourse.tile as tile
from concourse import bacc, mybir
from concourse.bass_utils import run_bass_kernel_spmd

F32 = mybir.dt.float32
BF16 = mybir.dt.bfloat16
AF = mybir.ActivationFunctionType

B = 4
C = 512
N = 4096  # pixels per sample (64*64)
NQ = 2048  # query columns per core
CT = 4  # channel tiles of 128
KT = 32  # key tiles of 128
QC = 4  # query chunks of 512 per core
GS = 16  # channels per group
EPS = 1e-5
SCALE = 1.0 / float(np.sqrt(C))

_CACHE: dict = {}
_PHASES = 3  # internal: truncate program for phase bisection (1=GN, 2=+qkv, 3=full)
_PIPELINE_LAG = 0  # O-matmuls trail S-matmuls by this many key tiles (0 = scheduler default, best measured)
_PSMM_BUFS = 3  # slots in the shared matmul PSUM pool (3 best measured; 4 was slower)


def _build_program(repeat: int = 1) -> "bass.Bass":
    key = (repeat, _PHASES, _PIPELINE_LAG, _PSMM_BUFS)
    if key in _CACHE:
        return _CACHE[key]
    nc = bacc.Bacc()

    x_d = nc.dram_tensor("x", [C, N], F32, kind="ExternalInput")
    wq_d = nc.dram_tensor("qwT", [C, C], BF16, kind="ExternalInput")
    wk_d = nc.dram_tensor("kwT", [C, C], BF16, kind="ExternalInput")
    wv_d = nc.dram_tensor("vwT", [C, C], BF16, kind="ExternalInput")
    wp_d = nc.dram_tensor("pwT", [C, C], BF16, kind="ExternalInput")
    qb_d = nc.dram_tensor("qb", [C, 1], F32, kind="ExternalInput")
    kb_d = nc.dram_tensor("kb", [C, 1], F32, kind="ExternalInput")
    vb_d = nc.dram_tensor("vb", [1, C], F32, kind="ExternalInput")
    pb_d = nc.dram_tensor("pb", [C, 1], F32, kind="ExternalInput")
    gnw_d = nc.dram_tensor("gnw", [C, 1], F32, kind="ExternalInput")
    gnb_d = nc.dram_tensor("gnb", [C, 1], F32, kind="ExternalInput")
    gmat_d = nc.dram_tensor("gmat", [128, 8], BF16, kind="ExternalInput")
    hmat_d = nc.dram_tensor("hmat", [8, 128], BF16, kind="ExternalInput")
    ones_d = nc.dram_tensor("ones128", [128, 1], BF16, kind="ExternalInput")
    y_d = nc.dram_tensor("y", [C, NQ], F32, kind="ExternalOutput")

    with tile.TileContext(nc) as tc, ExitStack() as ctx:

        def pool(name, bufs, space="SBUF"):
            return ctx.enter_context(tc.tile_pool(name=name, bufs=bufs, space=space))

        p_const = pool("const", 1)
        p_big = pool("big", 1)
        p_x = pool("xload", 2)
        p_st = pool("st", 2)
        p_sm = pool("sm", 16)
        p_e = pool("epool", 3)
        p_rs = pool("rs", 2)
        p_rb = pool("rb", 2)
        p_rin = pool("rin", 2)
        p_ob = pool("ob", 6)
        p_xr = pool("xr", 3)
        p_y = pool("ypool", 4)
        ps_mm = pool("psmm", _PSMM_BUFS, space="PSUM")
        ps_o = pool("pso", 4, space="PSUM")

        # ---- constants / weights ----
        gmat_sb = p_const.tile([128, 8], BF16, name="gmat_sb")
        nc.sync.dma_start(out=gmat_sb, in_=gmat_d[:, :])
        hmat_sb = p_const.tile([8, 128], BF16, name="hmat_sb")
        nc.sync.dma_start(out=hmat_sb, in_=hmat_d[:, :])
        ones_sb = p_const.tile([128, 1], BF16, name="ones_sb")
        nc.sync.dma_start(out=ones_sb, in_=ones_d[:, :])
        eps1_sb = p_const.tile([128, 1], F32, name="eps1_sb")
        nc.vector.memset(eps1_sb, 1.0 + EPS)

        def load_colvec(dram, nm):
            t = p_const.tile([128, CT, 1], F32, name=nm)
            nc.sync.dma_start(out=t, in_=dram.rearrange("(t p) o -> p t o", p=128))
            return t

        gnw_sb = load_colvec(gnw_d, "gnw_sb")
        gnb_sb = load_colvec(gnb_d, "gnb_sb")
        qb_sb = load_colvec(qb_d, "qb_sb")
        kb_sb = load_colvec(kb_d, "kb_sb")
        pb_sb = load_colvec(pb_d, "pb_sb")
        vb_sb = p_const.tile([128, C], F32, name="vb_sb")
        nc.sync.dma_start(out=vb_sb, in_=vb_d[:, :].to_broadcast([128, C]))

        def load_weight(dram, nm):
            t = p_const.tile([128, CT, C], BF16, name=nm)
            nc.sync.dma_start(out=t, in_=dram.rearrange("(t p) o -> p t o", p=128))
            return t

        wq_sb = load_weight(wq_d, "wq_sb")
        wk_sb = load_weight(wk_d, "wk_sb")
        wv_sb = load_weight(wv_d, "wv_sb")
        wp_sb = load_weight(wp_d, "wp_sb")

        # PE-side absorbers: one bare LDWEIGHTS per const-DMA so later real
        # matmuls never carry a DMA wait (walrus LDWEIGHTS allows 1 wait).
        for ap in (
            gmat_sb[:, :],
            hmat_sb[:, :],
            ones_sb[:, :],
            wq_sb[:, 0, 0:128],
            wk_sb[:, 0, 0:128],
            wv_sb[:, 0, 0:128],
            wp_sb[:, 0, 0:128],
            qb_sb[:, :, 0].bitcast(BF16),
            kb_sb[:, :, 0].bitcast(BF16),
            pb_sb[:, :, 0].bitcast(BF16),
            gnw_sb[:, :, 0].bitcast(BF16),
            gnb_sb[:, :, 0].bitcast(BF16),
            vb_sb[:, 0:64].bitcast(BF16),
        ):
            nc.tensor.ldweights(weights=ap)

        h_sb = p_big.tile([128, CT, N], BF16, name="h_sb")
        k_sb = p_big.tile([128, CT, N], BF16, name="k_sb")
        q_sb = p_big.tile([128, CT, NQ], BF16, name="q_sb")
        v_sb = p_big.tile([128, KT, C], BF16, name="v_sb")

        # optional on-device repeat loop (timing builds only)
        import contextlib
        loop_cm = tc.For_i(0, repeat, 1) if repeat > 1 else contextlib.nullcontext()
        with loop_cm:
            # ---- phase 1: GroupNorm -> h (bf16) ----
            for ct in range(CT):
                x_t = p_x.tile([128, N], F32, tag="x", name=f"x{ct}")
                nc.sync.dma_start(out=x_t, in_=x_d[ct * 128 : (ct + 1) * 128, :])
                xr = x_t.rearrange("p (n f) -> p n f", f=512)
                st = p_st.tile([128, 8, 6], F32, tag="st", name=f"st{ct}")
                for i in range(8):
                    nc.vector.bn_stats(out=st[:, i, :], in_=xr[:, i, :])
                mv = p_sm.tile([128, 2], F32, tag="sm", name=f"mv{ct}")
                nc.vector.bn_aggr(out=mv, in_=st)
                # ms = [mean, E[x^2]-1] per partition, bf16 (centering E[x^2]
                # around 1 keeps the bf16 rounding ~1e-5 absolute)
                m2 = p_sm.tile([128, 1], F32, tag="sm", name=f"m2{ct}")
                nc.vector.tensor_mul(out=m2, in0=mv[:, 0:1], in1=mv[:, 0:1])
                e2 = p_sm.tile([128, 1], F32, tag="sm", name=f"e2{ct}")
                nc.vector.tensor_add(out=e2, in0=m2, in1=mv[:, 1:2])
                ms = p_sm.tile([128, 2], BF16, tag="smf", name=f"ms{ct}")
                nc.vector.tensor_copy(out=ms[:, 0:1], in_=mv[:, 0:1])
                nc.vector.tensor_scalar_add(out=ms[:, 1:2], in0=e2, scalar1=-1.0)
                # cross-partition group combine: [128,2] -> [8,2] -> [128,2]
                g_ps = ps_mm.tile([8, 2], F32, tag="mm", name=f"gps{ct}")
                nc.tensor.matmul(g_ps, lhsT=gmat_sb, rhs=ms, start=True, stop=True)
                g_sb = p_sm.tile([8, 2], BF16, tag="smg", name=f"gsb{ct}")
                nc.scalar.copy(out=g_sb, in_=g_ps)
                b_ps = ps_mm.tile([128, 2], F32, tag="mm", name=f"bps{ct}")
                nc.tensor.matmul(b_ps, lhsT=hmat_sb, rhs=g_sb, start=True, stop=True)
                mb = p_sm.tile([128, 2], F32, tag="smb", name=f"mb{ct}")
                nc.scalar.copy(out=mb, in_=b_ps)
                # A = rstd * gn_w ; Bc = gn_b - mean * A
                t1 = p_sm.tile([128, 1], F32, tag="sm", name=f"t1{ct}")
                nc.vector.tensor_mul(out=t1, in0=mb[:, 0:1], in1=mb[:, 0:1])
                var = p_sm.tile([128, 1], F32, tag="sm", name=f"var{ct}")
                nc.vector.tensor_sub(out=var, in0=mb[:, 1:2], in1=t1)
                sd = p_sm.tile([128, 1], F32, tag="sm", name=f"sd{ct}")
                nc.scalar.activation(out=sd, in_=var, func=AF.Sqrt, bias=eps1_sb)
                rstd = p_sm.tile([128, 1], F32, tag="sm", name=f"rstd{ct}")
                nc.vector.reciprocal(out=rstd, in_=sd)
                a_c = p_sm.tile([128, 1], F32, tag="sm", name=f"ac{ct}")
                nc.vector.tensor_mul(out=a_c, in0=rstd, in1=gnw_sb[:, ct, :])
                t2 = p_sm.tile([128, 1], F32, tag="sm", name=f"t2{ct}")
                nc.vector.tensor_mul(out=t2, in0=mb[:, 0:1], in1=a_c)
                b_c = p_sm.tile([128, 1], F32, tag="sm", name=f"bc{ct}")
                nc.vector.tensor_sub(out=b_c, in0=gnb_sb[:, ct, :], in1=t2)
                nc.vector.tensor_scalar(
                    out=h_sb[:, ct, :],
                    in0=x_t,
                    scalar1=a_c,
                    scalar2=b_c,
                    op0=mybir.AluOpType.mult,
                    op1=mybir.AluOpType.add,
                )

            if _PHASES >= 2:
                # ---- phase 2: q, k, vT ----
                for oc in range(CT):
                    for nq in range(QC):
                        ps = ps_mm.tile([128, 512], F32, tag="mm", name=f"qp{oc}_{nq}")
                        for cc in range(CT):
                            nc.tensor.matmul(
                                ps,
                                lhsT=wq_sb[:, cc, oc * 128 : (oc + 1) * 128],
                                rhs=h_sb[:, cc, nq * 512 : (nq + 1) * 512],
                                start=(cc == 0),
                                stop=(cc == CT - 1),
                            )
                        nc.scalar.add(
                            out=q_sb[:, oc, nq * 512 : (nq + 1) * 512],
                            in_=ps,
                            add=qb_sb[:, oc, :],
                        )
                for oc in range(CT):
                    for nk in range(8):
                        ps = ps_mm.tile([128, 512], F32, tag="mm", name=f"kp{oc}_{nk}")
                        for cc in range(CT):
                            nc.tensor.matmul(
                                ps,
                                lhsT=wk_sb[:, cc, oc * 128 : (oc + 1) * 128],
                                rhs=h_sb[:, cc, nk * 512 : (nk + 1) * 512],
                                start=(cc == 0),
                                stop=(cc == CT - 1),
                            )
                        nc.scalar.add(
                            out=k_sb[:, oc, nk * 512 : (nk + 1) * 512],
                            in_=ps,
                            add=kb_sb[:, oc, :],
                        )
                for nt in range(KT):
                    ps = ps_mm.tile([128, 512], F32, tag="mm", name=f"vp{nt}")
                    for cc in range(CT):
                        nc.tensor.matmul(
                            ps,
                            lhsT=h_sb[:, cc, nt * 128 : (nt + 1) * 128],
                            rhs=wv_sb[:, cc, :],
                            start=(cc == 0),
                            stop=(cc == CT - 1),
                        )
                    nc.vector.tensor_add(out=v_sb[:, nt, :], in0=ps, in1=vb_sb)


            if _PHASES >= 3:
                # ---- phase 3: attention per 512-wide query chunk ----
                for qc in range(QC):
                    o_ps = [
                        ps_o.tile([128, 512], F32, tag="o", name=f"ops{qc}_{d}")
                        for d in range(CT)
                    ]
                    rs = p_rs.tile([128, 512], F32, tag="rs", name=f"rs{qc}")
                    # software pipeline: O-matmuls trail the S-matmuls by
                    # LAG iterations so the PE never waits on the ACT exp.
                    LAG = _PIPELINE_LAG
                    e_pipe = []

                    def emit_o(kt, e_tile):
                        for d in range(CT):
                            nc.tensor.matmul(
                                o_ps[d],
                                lhsT=v_sb[:, kt, d * 128 : (d + 1) * 128],
                                rhs=e_tile,
                                start=(kt == 0),
                                stop=(kt == KT - 1),
                            )

                    for kt in range(KT):
                        e_ps = ps_mm.tile([128, 512], F32, tag="mm", name=f"ep{qc}_{kt}")
                        for cc in range(CT):
                            nc.tensor.matmul(
                                e_ps,
                                lhsT=k_sb[:, cc, kt * 128 : (kt + 1) * 128],
                                rhs=q_sb[:, cc, qc * 512 : (qc + 1) * 512],
                                start=(cc == 0),
                                stop=(cc == CT - 1),
                            )
                        e_sb = p_e.tile([128, 512], BF16, tag="e", name=f"es{qc}_{kt}")
                        nc.scalar.activation(out=e_sb, in_=e_ps, func=AF.Exp, scale=SCALE)
                        if kt == 0:
                            nc.vector.tensor_copy(out=rs, in_=e_sb)
                        else:
                            nc.vector.tensor_add(out=rs, in0=rs, in1=e_sb)
                        e_pipe.append(e_sb)
                        if kt >= LAG:
                            emit_o(kt - LAG, e_pipe[kt - LAG])
                    for kt in range(KT - LAG, KT):
                        emit_o(kt, e_pipe[kt])
                    rs_bf = p_rb.tile([128, 512], BF16, tag="rsb", name=f"rsb{qc}")
                    nc.vector.tensor_copy(out=rs_bf, in_=rs)
                    rsum_ps = ps_mm.tile([1, 512], F32, tag="mm", name=f"rsum{qc}")
                    nc.tensor.matmul(rsum_ps, lhsT=ones_sb, rhs=rs_bf, start=True, stop=True)
                    rinv = p_rin.tile([1, 512], F32, tag="rin", name=f"rin{qc}")
                    nc.vector.reciprocal(out=rinv, in_=rsum_ps)
                    rb_sb = p_rb.tile([128, 512], F32, tag="rb", name=f"rb{qc}")
                    nc.gpsimd.partition_broadcast(rb_sb[:, :], rinv[:, :])
                    o_sb = []
                    for d in range(CT):
                        ot = p_ob.tile([128, 512], BF16, tag="ob", name=f"ob{qc}_{d}")
                        nc.vector.tensor_mul(out=ot, in0=o_ps[d], in1=rb_sb)
                        o_sb.append(ot)
                    for oc in range(CT):
                        y_ps = ps_mm.tile([128, 512], F32, tag="mm", name=f"yp{qc}_{oc}")
                        for d in range(CT):
                            nc.tensor.matmul(
                                y_ps,
                                lhsT=wp_sb[:, d, oc * 128 : (oc + 1) * 128],
                                rhs=o_sb[d],
                                start=(d == 0),
                                stop=(d == CT - 1),
                            )
                        xr_t = p_xr.tile([128, 512], F32, tag="xr", name=f"xr{qc}_{oc}")
                        nc.sync.dma_start(
                            out=xr_t,
                            in_=x_d[oc * 128 : (oc + 1) * 128, qc * 512 : (qc + 1) * 512],
                        )
                        t_sb = p_y.tile([128, 512], F32, tag="y", name=f"t{qc}_{oc}")
                        nc.scalar.add(out=t_sb, in_=y_ps, add=pb_sb[:, oc, :])
                        y2 = p_y.tile([128, 512], F32, tag="y", name=f"y2{qc}_{oc}")
                        nc.vector.tensor_add(out=y2, in0=t_sb, in1=xr_t)
                        nc.sync.dma_start(
                            out=y_d[oc * 128 : (oc + 1) * 128, qc * 512 : (qc + 1) * 512],
                            in_=y2,
                        )

    nc.finalize()
    _CACHE[key] = nc
    return nc


def _host_inputs(x, gn_w, gn_b, qw, qb, kw, kb, vw, vb, pw, pb):
    bf = ml_dtypes.bfloat16
    f32 = np.float32
    xf = np.asarray(x, f32).reshape(B, C, N)

    def wt(w):
        return np.ascontiguousarray(np.asarray(w, f32).T).astype(bf)

    gmat = np.zeros((128, 8), f32)
    for p in range(128):
        gmat[p, p // GS] = 1.0 / GS  # average the 16 per-partition means
    gmat = gmat.astype(bf)  # 1/16 is exact in bf16
    hmat = np.zeros((8, 128), f32)
    for p in range(128):
        hmat[p // GS, p] = 1.0
    hmat = hmat.astype(bf)
    shared = {
        "qwT": wt(qw),
        "kwT": wt(kw),
        "vwT": wt(vw),
        "pwT": wt(pw),
        "qb": np.asarray(qb, f32).reshape(C, 1),
        "kb": np.asarray(kb, f32).reshape(C, 1),
        "vb": np.asarray(vb, f32).reshape(1, C),
        "pb": np.asarray(pb, f32).reshape(C, 1),
        "gnw": np.asarray(gn_w, f32).reshape(C, 1),
        "gnb": np.asarray(gn_b, f32).reshape(C, 1),
        "gmat": gmat,
        "hmat": hmat,
        "ones128": np.ones((128, 1), bf),
    }
    in_maps = []
    for core in range(8):
        s, half = core // 2, core % 2
        xs = np.ascontiguousarray(np.roll(xf[s], -NQ * half, axis=1))
        in_maps.append({"x": xs, **shared})
    return in_maps


def kernel(x, gn_w, gn_b, qw, qb, kw, kb, vw, vb, pw, pb):
    nc = _build_program()
    in_maps = _host_inputs(x, gn_w, gn_b, qw, qb, kw, kb, vw, vb, pw, pb)
    res = run_bass_kernel_spmd(nc, in_maps, list(range(8)))
    outs = res.results
    y = np.empty((B, C, N), np.float32)
    for s in range(B):
        y[s][:, :NQ] = outs[2 * s]["y"]
        y[s][:, NQ:] = outs[2 * s + 1]["y"]
    return y.reshape(B, C, 64, 64)



# revision 9
# speedup vs baseline: 1.5136x; 1.5136x over previous
"""Trainium2 Bass kernel for AttnBlock (GroupNorm + 1x1-conv QKV + 4096x4096
attention + output projection + residual), B=4, C=512, H=W=64.

Sharding: 8 cores = 4 samples x 2 query-halves. Each core receives its
sample's x rolled so that "its" 2048 query columns are columns 0:2048 —
attention is invariant to key order, so one identical SPMD program serves
all 8 cores (no collectives, no per-core program specialization).

All heavy matmuls run in fp8e4m3 with MatmulPerfMode.DoubleRow (256-deep
contraction per instruction, measured 156 TF/s = 2x bf16 on HW). The 2e-2
relative-error budget is dominated by the fp32 residual x (the attention
branch is ~3% of output norm), so fp8 noise in the attention path is
diluted ~30x.

Per-core pipeline (layouts [channel-on-partition, pixel-on-free] unless
noted):
  1. GroupNorm(32 groups): bn_stats per partition, cross-partition group
     combine via a tiny matmul with a group-selector matrix, normalize to
     h (fp8).
  2. q = qw@h (2048 cols), k = kw@h (4096 cols), vT = h^T@wv^T (v
     transposed so the attention O-matmul contracts keys on partitions).
     All DoubleRow over channel-tile pairs.
  3. Per 512-wide query chunk: S^T tiles = k^T q (keys on partitions),
     exp(s - 2.5) on ACT (shift keeps fp8 e < 240=inf; softmax-invariant),
     e written fp8 into a 4-slot ring so kt-pairs form DoubleRow rhs.
     O accumulated over 16 kt-pairs; softmax row-sums via a ones-column
     DoubleRow matmul into PSUM (no DVE rowsum); then O * (1/rowsum), the
     output projection, +bias +residual via one scalar_tensor_tensor.
"""

import sys

import numpy as np

try:
    import concourse.bass as bass
except ImportError:  # harness environments differ in sys.path
    sys.path.insert(0, "/opt/trn_rl_repo")
    import concourse.bass as bass

from contextlib import ExitStack

import ml_dtypes

import concourse.tile as tile
from concourse import bacc, mybir
from concourse.bass_utils import run_bass_kernel_spmd

F32 = mybir.dt.float32
BF16 = mybir.dt.bfloat16
FP8 = mybir.dt.float8e4
AF = mybir.ActivationFunctionType
DR = mybir.MatmulPerfMode.DoubleRow
ALU = mybir.AluOpType

B = 4
C = 512
N = 4096  # pixels per sample (64*64)
NQ = 2048  # query columns per core
CT = 4  # channel tiles of 128
KT = 32  # key tiles of 128
QC = 4  # query chunks of 512 per core
GS = 16  # channels per group
EPS = 1e-5
SCALE = 1.0 / float(np.sqrt(C))
EXP_BIAS = -2.5  # exp(s+bias): keeps fp8 e-max ~129 << 240 (e4m3 inf)

_CACHE: dict = {}
_PHASES = 3  # internal: truncate program for phase bisection (1=GN, 2=+qkv, 3=full)


def _build_program(repeat: int = 1) -> "bass.Bass":
    key = (repeat, _PHASES)
    if key in _CACHE:
        return _CACHE[key]
    nc = bacc.Bacc()

    x_d = nc.dram_tensor("x", [C, N], F32, kind="ExternalInput")
    wq_d = nc.dram_tensor("qwT", [C, C], FP8, kind="ExternalInput")
    wk_d = nc.dram_tensor("kwT", [C, C], FP8, kind="ExternalInput")
    wv_d = nc.dram_tensor("vwT", [C, C], FP8, kind="ExternalInput")
    wp_d = nc.dram_tensor("pwT", [C, C], FP8, kind="ExternalInput")
    qb_d = nc.dram_tensor("qb", [C, 1], F32, kind="ExternalInput")
    kb_d = nc.dram_tensor("kb", [C, 1], F32, kind="ExternalInput")
    vb_d = nc.dram_tensor("vb", [1, C], F32, kind="ExternalInput")
    pb_d = nc.dram_tensor("pb", [C, 1], F32, kind="ExternalInput")
    gnw_d = nc.dram_tensor("gnw", [C, 1], F32, kind="ExternalInput")
    gnb_d = nc.dram_tensor("gnb", [C, 1], F32, kind="ExternalInput")
    gmat_d = nc.dram_tensor("gmat", [128, 8], BF16, kind="ExternalInput")
    hmat_d = nc.dram_tensor("hmat", [8, 128], BF16, kind="ExternalInput")
    ones_d = nc.dram_tensor("ones2", [128, 32], FP8, kind="ExternalInput")
    y_d = nc.dram_tensor("y", [C, NQ], F32, kind="ExternalOutput")

    with tile.TileContext(nc) as tc, ExitStack() as ctx:

        def pool(name, bufs, space="SBUF"):
            return ctx.enter_context(tc.tile_pool(name=name, bufs=bufs, space=space))

        p_const = pool("const", 1)
        p_big = pool("big", 1)
        p_x = pool("xload", 2)
        p_st = pool("st", 2)
        p_sm = pool("sm", 16)
        p_e = pool("epool", 2)
        p_rin = pool("rin", 2)
        p_rb = pool("rb", 2)
        p_ob = pool("ob", 2)
        p_xr = pool("xr", 3)
        p_y = pool("ypool", 4)
        ps_a = pool("psa", 1, space="PSUM")  # tags: s(3), rs(1), o(4) = 8 banks

        # ---- constants / weights ----
        gmat_sb = p_const.tile([128, 8], BF16, tag="c0", name="gmat_sb")
        nc.sync.dma_start(out=gmat_sb, in_=gmat_d[:, :])
        hmat_sb = p_const.tile([8, 128], BF16, tag="c1", name="hmat_sb")
        nc.sync.dma_start(out=hmat_sb, in_=hmat_d[:, :])
        # DR stationary must be >=16 wide: 16 ones-columns -> 16 identical
        # rowsum rows in PSUM; row 0 is used.
        ones_sb = p_const.tile([128, 2, 16], FP8, tag="c2", name="ones_sb")
        nc.sync.dma_start(out=ones_sb, in_=ones_d.rearrange("p (t o) -> p t o", o=16))
        eps1_sb = p_const.tile([128, 1], F32, tag="c3", name="eps1_sb")
        nc.vector.memset(eps1_sb, 1.0 + EPS)
        ebias_sb = p_const.tile([128, 1], F32, tag="c3b", name="ebias_sb")
        nc.vector.memset(ebias_sb, EXP_BIAS)

        def load_colvec(dram, nm, tg):
            t = p_const.tile([128, CT, 1], F32, tag=tg, name=nm)
            nc.sync.dma_start(out=t, in_=dram.rearrange("(t p) o -> p t o", p=128))
            return t

        gnw_sb = load_colvec(gnw_d, "gnw_sb", "c4")
        gnb_sb = load_colvec(gnb_d, "gnb_sb", "c5")
        qb_sb = load_colvec(qb_d, "qb_sb", "c6")
        kb_sb = load_colvec(kb_d, "kb_sb", "c7")
        pb_sb = load_colvec(pb_d, "pb_sb", "c8")
        vb_sb = p_const.tile([128, C], F32, tag="c9", name="vb_sb")
        nc.sync.dma_start(out=vb_sb, in_=vb_d[:, :].to_broadcast([128, C]))

        def load_weight(dram, nm, tg):
            t = p_const.tile([128, CT, C], FP8, tag=tg, name=nm)
            nc.sync.dma_start(out=t, in_=dram.rearrange("(t p) o -> p t o", p=128))
            return t

        wq_sb = load_weight(wq_d, "wq_sb", "w0")
        wk_sb = load_weight(wk_d, "wk_sb", "w1")
        wv_sb = load_weight(wv_d, "wv_sb", "w2")
        wp_sb = load_weight(wp_d, "wp_sb", "w3")

        # PE-side absorbers: one bare LDWEIGHTS per const-DMA so later real
        # matmuls never carry a DMA wait (walrus LDWEIGHTS allows 1 wait).
        for ap in (
            gmat_sb[:, :],
            hmat_sb[:, :],
            ones_sb[:, 0, :],
            wq_sb[:, 0, 0:128],
            wk_sb[:, 0, 0:128],
            wv_sb[:, 0, 0:128],
            wp_sb[:, 0, 0:128],
            qb_sb[:, :, 0].bitcast(BF16),
            kb_sb[:, :, 0].bitcast(BF16),
            pb_sb[:, :, 0].bitcast(BF16),
            gnw_sb[:, :, 0].bitcast(BF16),
            gnb_sb[:, :, 0].bitcast(BF16),
            vb_sb[:, 0:64].bitcast(BF16),
        ):
            nc.tensor.ldweights(weights=ap)

        h_sb = p_big.tile([128, CT, N], FP8, tag="h", name="h_sb")
        k_sb = p_big.tile([128, CT, N], FP8, tag="k", name="k_sb")
        q_sb = p_big.tile([128, CT, NQ], FP8, tag="q", name="q_sb")
        v_sb = p_big.tile([128, KT, C], FP8, tag="v", name="v_sb")

        # optional on-device repeat loop (timing builds only)
        import contextlib

        loop_cm = tc.For_i(0, repeat, 1) if repeat > 1 else contextlib.nullcontext()
        with loop_cm:
            # ---- phase 1: GroupNorm -> h (fp8) ----
            for ct in range(CT):
                x_t = p_x.tile([128, N], F32, tag="x", name=f"x{ct}")
                nc.sync.dma_start(out=x_t, in_=x_d[ct * 128 : (ct + 1) * 128, :])
                xr = x_t.rearrange("p (n f) -> p n f", f=512)
                st = p_st.tile([128, 8, 6], F32, tag="st", name=f"st{ct}")
                for i in range(8):
                    nc.vector.bn_stats(out=st[:, i, :], in_=xr[:, i, :])
                mv = p_sm.tile([128, 2], F32, tag="sm", name=f"mv{ct}")
                nc.vector.bn_aggr(out=mv, in_=st)
                # ms = [mean, E[x^2]-1] per partition, bf16 (centering E[x^2]
                # around 1 keeps the bf16 rounding ~1e-5 absolute)
                m2 = p_sm.tile([128, 1], F32, tag="sm", name=f"m2{ct}")
                nc.vector.tensor_mul(out=m2, in0=mv[:, 0:1], in1=mv[:, 0:1])
                e2 = p_sm.tile([128, 1], F32, tag="sm", name=f"e2{ct}")
                nc.vector.tensor_add(out=e2, in0=m2, in1=mv[:, 1:2])
                ms = p_sm.tile([128, 2], BF16, tag="smf", name=f"ms{ct}")
                nc.vector.tensor_copy(out=ms[:, 0:1], in_=mv[:, 0:1])
                nc.vector.tensor_scalar_add(out=ms[:, 1:2], in0=e2, scalar1=-1.0)
                # cross-partition group combine: [128,2] -> [8,2] -> [128,2]
                g_ps = ps_a.tile([8, 2], F32, tag="s", bufs=3, name=f"gps{ct}")
                nc.tensor.matmul(g_ps, lhsT=gmat_sb, rhs=ms, start=True, stop=True)
                g_sb = p_sm.tile([8, 2], BF16, tag="smg", name=f"gsb{ct}")
                nc.scalar.copy(out=g_sb, in_=g_ps)
                b_ps = ps_a.tile([128, 2], F32, tag="s", bufs=3, name=f"bps{ct}")
                nc.tensor.matmul(b_ps, lhsT=hmat_sb, rhs=g_sb, start=True, stop=True)
                mb = p_sm.tile([128, 2], F32, tag="smb", name=f"mb{ct}")
                nc.scalar.copy(out=mb, in_=b_ps)
                # A = rstd * gn_w ; Bc = gn_b - mean * A
                t1 = p_sm.tile([128, 1], F32, tag="sm", name=f"t1{ct}")
                nc.vector.tensor_mul(out=t1, in0=mb[:, 0:1], in1=mb[:, 0:1])
                var = p_sm.tile([128, 1], F32, tag="sm", name=f"var{ct}")
                nc.vector.tensor_sub(out=var, in0=mb[:, 1:2], in1=t1)
                sd = p_sm.tile([128, 1], F32, tag="sm", name=f"sd{ct}")
                nc.scalar.activation(out=sd, in_=var, func=AF.Sqrt, bias=eps1_sb)
                rstd = p_sm.tile([128, 1], F32, tag="sm", name=f"rstd{ct}")
                nc.vector.reciprocal(out=rstd, in_=sd)
                a_c = p_sm.tile([128, 1], F32, tag="sm", name=f"ac{ct}")
                nc.vector.tensor_mul(out=a_c, in0=rstd, in1=gnw_sb[:, ct, :])
                t2 = p_sm.tile([128, 1], F32, tag="sm", name=f"t2{ct}")
                nc.vector.tensor_mul(out=t2, in0=mb[:, 0:1], in1=a_c)
                b_c = p_sm.tile([128, 1], F32, tag="sm", name=f"bc{ct}")
                nc.vector.tensor_sub(out=b_c, in0=gnb_sb[:, ct, :], in1=t2)
                nc.vector.tensor_scalar(
                    out=h_sb[:, ct, :],
                    in0=x_t,
                    scalar1=a_c,
                    scalar2=b_c,
                    op0=ALU.mult,
                    op1=ALU.add,
                )

            if _PHASES >= 2:
                # ---- phase 2: q, k, vT (fp8 DoubleRow over cc pairs) ----
                nalt = 0
                for oc in range(CT):
                    for nq in range(QC):
                        ps = ps_a.tile([128, 512], F32, tag="s", bufs=3, name=f"qp{oc}_{nq}")
                        for cp in range(2):
                            nc.tensor.matmul(
                                ps,
                                lhsT=wq_sb[:, 2 * cp : 2 * cp + 2, oc * 128 : (oc + 1) * 128],
                                rhs=h_sb[:, 2 * cp : 2 * cp + 2, nq * 512 : (nq + 1) * 512],
                                start=(cp == 0),
                                stop=(cp == 1),
                                perf_mode=DR,
                            )
                        dst = q_sb[:, oc, nq * 512 : (nq + 1) * 512]
                        if nalt % 2 == 0:
                            nc.scalar.add(out=dst, in_=ps, add=qb_sb[:, oc, :])
                        else:
                            nc.vector.tensor_scalar_add(out=dst, in0=ps, scalar1=qb_sb[:, oc, :])
                        nalt += 1
                for oc in range(CT):
                    for nk in range(8):
                        ps = ps_a.tile([128, 512], F32, tag="s", bufs=3, name=f"kp{oc}_{nk}")
                        for cp in range(2):
                            nc.tensor.matmul(
                                ps,
                                lhsT=wk_sb[:, 2 * cp : 2 * cp + 2, oc * 128 : (oc + 1) * 128],
                                rhs=h_sb[:, 2 * cp : 2 * cp + 2, nk * 512 : (nk + 1) * 512],
                                start=(cp == 0),
                                stop=(cp == 1),
                                perf_mode=DR,
                            )
                        dst = k_sb[:, oc, nk * 512 : (nk + 1) * 512]
                        if nalt % 2 == 0:
                            nc.scalar.add(out=dst, in_=ps, add=kb_sb[:, oc, :])
                        else:
                            nc.vector.tensor_scalar_add(out=dst, in0=ps, scalar1=kb_sb[:, oc, :])
                        nalt += 1
                for nt in range(KT):
                    ps = ps_a.tile([128, 512], F32, tag="s", bufs=3, name=f"vp{nt}")
                    for cp in range(2):
                        nc.tensor.matmul(
                            ps,
                            lhsT=h_sb[:, 2 * cp : 2 * cp + 2, nt * 128 : (nt + 1) * 128],
                            rhs=wv_sb[:, 2 * cp : 2 * cp + 2, :],
                            start=(cp == 0),
                            stop=(cp == 1),
                            perf_mode=DR,
                        )
                    # v bias is per-free-dim here (vT layout): DVE add
                    nc.vector.tensor_add(out=v_sb[:, nt, :], in0=ps, in1=vb_sb)

            if _PHASES >= 3:
                # ---- phase 3: attention per 512-wide query chunk ----
                for qc in range(QC):
                    o_ps = [
                        ps_a.tile([128, 512], F32, tag="o", bufs=4, name=f"ops{qc}_{d}")
                        for d in range(CT)
                    ]
                    rs_ps = ps_a.tile([16, 512], F32, tag="rs", bufs=1, name=f"rs{qc}")
                    e_roll = p_e.tile([128, 4, 512], FP8, tag="e", name=f"e{qc}")

                    def emit_pair(j):
                        sl = (2 * j) % 4
                        e_pair = e_roll[:, sl : sl + 2, :]
                        nc.tensor.matmul(
                            rs_ps,
                            lhsT=ones_sb,
                            rhs=e_pair,
                            start=(j == 0),
                            stop=(j == KT // 2 - 1),
                            perf_mode=DR,
                        )
                        for d in range(CT):
                            nc.tensor.matmul(
                                o_ps[d],
                                lhsT=v_sb[:, 2 * j : 2 * j + 2, d * 128 : (d + 1) * 128],
                                rhs=e_pair,
                                start=(j == 0),
                                stop=(j == KT // 2 - 1),
                                perf_mode=DR,
                            )

                    pending = None
                    for kt in range(KT):
                        s_ps = ps_a.tile([128, 512], F32, tag="s", bufs=3, name=f"sp{qc}_{kt}")
                        for cp in range(2):
                            nc.tensor.matmul(
                                s_ps,
                                lhsT=k_sb[:, 2 * cp : 2 * cp + 2, kt * 128 : (kt + 1) * 128],
                                rhs=q_sb[:, 2 * cp : 2 * cp + 2, qc * 512 : (qc + 1) * 512],
                                start=(cp == 0),
                                stop=(cp == 1),
                                perf_mode=DR,
                            )
                        nc.scalar.activation(
                            out=e_roll[:, kt % 4, :],
                            in_=s_ps,
                            func=AF.Exp,
                            scale=SCALE,
                            bias=ebias_sb,
                        )
                        if kt % 2 == 1:
                            # O-matmuls trail the S-matmuls by one kt-pair so
                            # the PE never waits on the ACT exp.
                            if pending is not None:
                                emit_pair(pending)
                            pending = kt // 2
                    emit_pair(pending)
                    rinv = p_rin.tile([1, 512], F32, tag="rin", name=f"rin{qc}")
                    nc.vector.reciprocal(out=rinv, in_=rs_ps[0:1, :])
                    rb_sb = p_rb.tile([128, 512], F32, tag="rb", name=f"rb{qc}")
                    nc.gpsimd.partition_broadcast(rb_sb[:, :], rinv[:, :])
                    o_all = p_ob.tile([128, CT, 512], FP8, tag="ob", name=f"ob{qc}")
                    for d in range(CT):
                        nc.vector.tensor_mul(out=o_all[:, d, :], in0=o_ps[d], in1=rb_sb)
                    for oc in range(CT):
                        y_ps = ps_a.tile([128, 512], F32, tag="s", bufs=3, name=f"yp{qc}_{oc}")
                        for dp in range(2):
                            nc.tensor.matmul(
                                y_ps,
                                lhsT=wp_sb[:, 2 * dp : 2 * dp + 2, oc * 128 : (oc + 1) * 128],
                                rhs=o_all[:, 2 * dp : 2 * dp + 2, :],
                                start=(dp == 0),
                                stop=(dp == 1),
                                perf_mode=DR,
                            )
                        xr_t = p_xr.tile([128, 512], F32, tag="xr", name=f"xr{qc}_{oc}")
                        nc.sync.dma_start(
                            out=xr_t,
                            in_=x_d[oc * 128 : (oc + 1) * 128, qc * 512 : (qc + 1) * 512],
                        )
                        y2 = p_y.tile([128, 512], F32, tag="y", name=f"y2{qc}_{oc}")
                        # y2 = (y_ps + pb) + x  in one DVE op
                        nc.vector.scalar_tensor_tensor(
                            out=y2,
                            in0=y_ps,
                            scalar=pb_sb[:, oc, :],
                            in1=xr_t,
                            op0=ALU.add,
                            op1=ALU.add,
                        )
                        nc.sync.dma_start(
                            out=y_d[oc * 128 : (oc + 1) * 128, qc * 512 : (qc + 1) * 512],
                            in_=y2,
                        )

    nc.finalize()
    _CACHE[key] = nc
    return nc


def _host_inputs(x, gn_w, gn_b, qw, qb, kw, kb, vw, vb, pw, pb):
    f8 = ml_dtypes.float8_e4m3
    f32 = np.float32
    bf = ml_dtypes.bfloat16
    xf = np.asarray(x, f32).reshape(B, C, N)

    def wt(w):
        return np.ascontiguousarray(np.asarray(w, f32).T).astype(f8)

    gmat = np.zeros((128, 8), f32)
    for p in range(128):
        gmat[p, p // GS] = 1.0 / GS  # average the 16 per-partition means
    gmat = gmat.astype(bf)  # 1/16 is exact in bf16
    hmat = np.zeros((8, 128), f32)
    for p in range(128):
        hmat[p // GS, p] = 1.0
    hmat = hmat.astype(bf)
    shared = {
        "qwT": wt(qw),
        "kwT": wt(kw),
        "vwT": wt(vw),
        "pwT": wt(pw),
        "qb": np.asarray(qb, f32).reshape(C, 1),
        "kb": np.asarray(kb, f32).reshape(C, 1),
        "vb": np.asarray(vb, f32).reshape(1, C),
        "pb": np.asarray(pb, f32).reshape(C, 1),
        "gnw": np.asarray(gn_w, f32).reshape(C, 1),
        "gnb": np.asarray(gn_b, f32).reshape(C, 1),
        "gmat": gmat,
        "hmat": hmat,
        "ones2": np.ones((128, 32), f8),
    }
    in_maps = []
    for core in range(8):
        s, half = core // 2, core % 2
        xs = np.ascontiguousarray(np.roll(xf[s], -NQ * half, axis=1))
        in_maps.append({"x": xs, **shared})
    return in_maps


def kernel(x, gn_w, gn_b, qw, qb, kw, kb, vw, vb, pw, pb):
    nc = _build_program()
    in_maps = _host_inputs(x, gn_w, gn_b, qw, qb, kw, kb, vw, vb, pw, pb)
    res = run_bass_kernel_spmd(nc, in_maps, list(range(8)))
    outs = res.results
    y = np.empty((B, C, N), np.float32)
    for s in range(B):
        y[s][:, :NQ] = outs[2 * s]["y"]
        y[s][:, NQ:] = outs[2 * s + 1]["y"]
    return y.reshape(B, C, 64, 64)


# revision 16
# speedup vs baseline: 1.6406x; 1.0839x over previous
"""Trainium2 Bass kernel for AttnBlock (GroupNorm + 1x1-conv QKV + 4096x4096
attention + output projection + residual), B=4, C=512, H=W=64.

Sharding: 8 cores = 4 samples x 2 query-halves. Each core receives its
sample's x rolled so that "its" 2048 query columns are columns 0:2048 —
attention is invariant to key order, so one identical SPMD program serves
all 8 cores (no collectives, no per-core program specialization).

All heavy matmuls run in fp8e4m3 with MatmulPerfMode.DoubleRow (256-deep
contraction per instruction, measured 156 TF/s = 2x bf16 on HW). The 2e-2
relative-error budget is dominated by the fp32 residual x (the attention
branch is ~3% of output norm), so fp8 noise in the attention path is
diluted ~30x.

Per-core pipeline (layouts [channel-on-partition, pixel-on-free] unless
noted):
  1. GroupNorm(32 groups): bn_stats per partition, cross-partition group
     combine via a tiny matmul with a group-selector matrix, normalize to
     h (fp8).
  2. q = qw@h (2048 cols), k = kw@h (4096 cols), vT = h^T@wv^T (v
     transposed so the attention O-matmul contracts keys on partitions).
     All DoubleRow over channel-tile pairs.
  3. Per 512-wide query chunk: S^T tiles = k^T q (keys on partitions),
     exp(s - 2.5) on ACT (shift keeps fp8 e < 240=inf; softmax-invariant),
     e written fp8 into a 4-slot ring so kt-pairs form DoubleRow rhs.
     O accumulated over 16 kt-pairs; softmax row-sums via a ones-column
     DoubleRow matmul into PSUM (no DVE rowsum); then O * (1/rowsum), the
     output projection, +bias +residual via one scalar_tensor_tensor.
"""

import sys

import numpy as np

try:
    import concourse.bass as bass
except ImportError:  # harness environments differ in sys.path
    sys.path.insert(0, "/opt/trn_rl_repo")
    import concourse.bass as bass

from contextlib import ExitStack

import ml_dtypes

import concourse.tile as tile
from concourse import bacc, mybir
from concourse.bass_utils import run_bass_kernel_spmd

F32 = mybir.dt.float32
BF16 = mybir.dt.bfloat16
FP8 = mybir.dt.float8e4
AF = mybir.ActivationFunctionType
DR = mybir.MatmulPerfMode.DoubleRow
ALU = mybir.AluOpType

B = 4
C = 512
N = 4096  # pixels per sample (64*64)
NQ = 2048  # query columns per core
CT = 4  # channel tiles of 128
KT = 32  # key tiles of 128
QC = 4  # query chunks of 512 per core
GS = 16  # channels per group
EPS = 1e-5
SCALE = 1.0 / float(np.sqrt(C))
EXP_BIAS = -2.5  # exp(s+bias): keeps fp8 e-max ~129 << 240 (e4m3 inf)

_CACHE: dict = {}
_PHASES = 3  # internal: truncate program for phase bisection (1=GN, 2=+qkv, 3=full)


def _build_program(repeat: int = 1) -> "bass.Bass":
    key = (repeat, _PHASES)
    if key in _CACHE:
        return _CACHE[key]
    nc = bacc.Bacc()

    x_d = nc.dram_tensor("x", [C, N], F32, kind="ExternalInput")
    wq_d = nc.dram_tensor("qwT", [C, C], FP8, kind="ExternalInput")
    wk_d = nc.dram_tensor("kwT", [C, C], FP8, kind="ExternalInput")
    wv_d = nc.dram_tensor("vwT", [C, C], FP8, kind="ExternalInput")
    wp_d = nc.dram_tensor("pwT", [C, C], FP8, kind="ExternalInput")
    qb_d = nc.dram_tensor("qb", [C, 1], F32, kind="ExternalInput")
    kb_d = nc.dram_tensor("kb", [C, 1], F32, kind="ExternalInput")
    vb_d = nc.dram_tensor("vb", [1, C], F32, kind="ExternalInput")
    pb_d = nc.dram_tensor("pb", [C, 1], F32, kind="ExternalInput")
    gnw_d = nc.dram_tensor("gnw", [C, 1], F32, kind="ExternalInput")
    gnb_d = nc.dram_tensor("gnb", [C, 1], F32, kind="ExternalInput")
    gmat_d = nc.dram_tensor("gmat", [128, 8], BF16, kind="ExternalInput")
    hmat_d = nc.dram_tensor("hmat", [8, 128], BF16, kind="ExternalInput")
    ones_d = nc.dram_tensor("ones2", [128, 32], FP8, kind="ExternalInput")
    y_d = nc.dram_tensor("y", [C, NQ], F32, kind="ExternalOutput")

    with tile.TileContext(nc) as tc, ExitStack() as ctx:

        def pool(name, bufs, space="SBUF"):
            return ctx.enter_context(tc.tile_pool(name=name, bufs=bufs, space=space))

        p_const = pool("const", 1)
        p_big = pool("big", 1)
        p_x = pool("xload", 2)
        p_st = pool("st", 2)
        p_sm = pool("sm", 16)
        p_e = pool("epool", 2)
        p_rin = pool("rin", 2)
        p_rb = pool("rb", 2)
        p_ob = pool("ob", 2)
        p_xr = pool("xr", 4)  # 4 wide residual tiles live concurrently
        p_y = pool("ypool", 4)  # 4 wide y stages live concurrently
        ps_a = pool("psa", 1, space="PSUM")  # tags: s(3), rs(1), o(4) = 8 banks

        # ---- constants / weights ----
        gmat_sb = p_const.tile([128, 8], BF16, tag="c0", name="gmat_sb")
        nc.sync.dma_start(out=gmat_sb, in_=gmat_d[:, :])
        hmat_sb = p_const.tile([8, 128], BF16, tag="c1", name="hmat_sb")
        nc.sync.dma_start(out=hmat_sb, in_=hmat_d[:, :])
        # DR stationary must be >=16 wide: 16 ones-columns -> 16 identical
        # rowsum rows in PSUM; row 0 is used.
        ones_sb = p_const.tile([128, 2, 16], FP8, tag="c2", name="ones_sb")
        nc.sync.dma_start(out=ones_sb, in_=ones_d.rearrange("p (t o) -> p t o", o=16))
        eps1_sb = p_const.tile([128, 1], F32, tag="c3", name="eps1_sb")
        nc.vector.memset(eps1_sb, 1.0 + EPS)
        ebias_sb = p_const.tile([128, 1], F32, tag="c3b", name="ebias_sb")
        nc.vector.memset(ebias_sb, EXP_BIAS)

        def load_colvec(dram, nm, tg):
            t = p_const.tile([128, CT, 1], F32, tag=tg, name=nm)
            nc.sync.dma_start(out=t, in_=dram.rearrange("(t p) o -> p t o", p=128))
            return t

        gnw_sb = load_colvec(gnw_d, "gnw_sb", "c4")
        gnb_sb = load_colvec(gnb_d, "gnb_sb", "c5")
        qb_sb = load_colvec(qb_d, "qb_sb", "c6")
        kb_sb = load_colvec(kb_d, "kb_sb", "c7")
        pb_sb = load_colvec(pb_d, "pb_sb", "c8")
        vb_sb = p_const.tile([128, C], F32, tag="c9", name="vb_sb")
        nc.sync.dma_start(out=vb_sb, in_=vb_d[:, :].to_broadcast([128, C]))

        def load_weight(dram, nm, tg):
            t = p_const.tile([128, CT, C], FP8, tag=tg, name=nm)
            nc.sync.dma_start(out=t, in_=dram.rearrange("(t p) o -> p t o", p=128))
            return t

        wq_sb = load_weight(wq_d, "wq_sb", "w0")
        wk_sb = load_weight(wk_d, "wk_sb", "w1")
        wv_sb = load_weight(wv_d, "wv_sb", "w2")
        wp_sb = load_weight(wp_d, "wp_sb", "w3")

        # PE-side absorbers: one bare LDWEIGHTS per const-DMA so later real
        # matmuls never carry a DMA wait (walrus LDWEIGHTS allows 1 wait).
        for ap in (
            gmat_sb[:, :],
            hmat_sb[:, :],
            ones_sb[:, 0, :],
            wq_sb[:, 0, 0:128],
            wk_sb[:, 0, 0:128],
            wv_sb[:, 0, 0:128],
            wp_sb[:, 0, 0:128],
            qb_sb[:, :, 0].bitcast(BF16),
            kb_sb[:, :, 0].bitcast(BF16),
            pb_sb[:, :, 0].bitcast(BF16),
            gnw_sb[:, :, 0].bitcast(BF16),
            gnb_sb[:, :, 0].bitcast(BF16),
            vb_sb[:, 0:64].bitcast(BF16),
        ):
            nc.tensor.ldweights(weights=ap)

        h_sb = p_big.tile([128, CT, N], FP8, tag="h", name="h_sb")
        k_sb = p_big.tile([128, CT, N], FP8, tag="k", name="k_sb")
        q_sb = p_big.tile([128, CT, NQ], FP8, tag="q", name="q_sb")
        v_sb = p_big.tile([128, KT, C], FP8, tag="v", name="v_sb")

        # optional on-device repeat loop (timing builds only)
        import contextlib

        loop_cm = tc.For_i(0, repeat, 1) if repeat > 1 else contextlib.nullcontext()
        with loop_cm:
            # ---- phase 1: GroupNorm -> h (fp8) ----
            for ct in range(CT):
                x_t = p_x.tile([128, N], F32, tag="x", name=f"x{ct}")
                nc.sync.dma_start(out=x_t, in_=x_d[ct * 128 : (ct + 1) * 128, :])
                xr = x_t.rearrange("p (n f) -> p n f", f=512)
                st = p_st.tile([128, 8, 6], F32, tag="st", name=f"st{ct}")
                for i in range(8):
                    nc.vector.bn_stats(out=st[:, i, :], in_=xr[:, i, :])
                mv = p_sm.tile([128, 2], F32, tag="sm", name=f"mv{ct}")
                nc.vector.bn_aggr(out=mv, in_=st)
                # ms = [mean, E[x^2]-1] per partition, bf16 (centering E[x^2]
                # around 1 keeps the bf16 rounding ~1e-5 absolute)
                m2 = p_sm.tile([128, 1], F32, tag="sm", name=f"m2{ct}")
                nc.vector.tensor_mul(out=m2, in0=mv[:, 0:1], in1=mv[:, 0:1])
                e2 = p_sm.tile([128, 1], F32, tag="sm", name=f"e2{ct}")
                nc.vector.tensor_add(out=e2, in0=m2, in1=mv[:, 1:2])
                ms = p_sm.tile([128, 2], BF16, tag="smf", name=f"ms{ct}")
                nc.vector.tensor_copy(out=ms[:, 0:1], in_=mv[:, 0:1])
                nc.vector.tensor_scalar_add(out=ms[:, 1:2], in0=e2, scalar1=-1.0)
                # cross-partition group combine: [128,2] -> [8,2] -> [128,2]
                g_ps = ps_a.tile([8, 2], F32, tag="s", bufs=3, name=f"gps{ct}")
                nc.tensor.matmul(g_ps, lhsT=gmat_sb, rhs=ms, start=True, stop=True)
                g_sb = p_sm.tile([8, 2], BF16, tag="smg", name=f"gsb{ct}")
                nc.scalar.copy(out=g_sb, in_=g_ps)
                b_ps = ps_a.tile([128, 2], F32, tag="s", bufs=3, name=f"bps{ct}")
                nc.tensor.matmul(b_ps, lhsT=hmat_sb, rhs=g_sb, start=True, stop=True)
                mb = p_sm.tile([128, 2], F32, tag="smb", name=f"mb{ct}")
                nc.scalar.copy(out=mb, in_=b_ps)
                # A = rstd * gn_w ; Bc = gn_b - mean * A
                t1 = p_sm.tile([128, 1], F32, tag="sm", name=f"t1{ct}")
                nc.vector.tensor_mul(out=t1, in0=mb[:, 0:1], in1=mb[:, 0:1])
                var = p_sm.tile([128, 1], F32, tag="sm", name=f"var{ct}")
                nc.vector.tensor_sub(out=var, in0=mb[:, 1:2], in1=t1)
                sd = p_sm.tile([128, 1], F32, tag="sm", name=f"sd{ct}")
                nc.scalar.activation(out=sd, in_=var, func=AF.Sqrt, bias=eps1_sb)
                rstd = p_sm.tile([128, 1], F32, tag="sm", name=f"rstd{ct}")
                nc.vector.reciprocal(out=rstd, in_=sd)
                a_c = p_sm.tile([128, 1], F32, tag="sm", name=f"ac{ct}")
                nc.vector.tensor_mul(out=a_c, in0=rstd, in1=gnw_sb[:, ct, :])
                t2 = p_sm.tile([128, 1], F32, tag="sm", name=f"t2{ct}")
                nc.vector.tensor_mul(out=t2, in0=mb[:, 0:1], in1=a_c)
                b_c = p_sm.tile([128, 1], F32, tag="sm", name=f"bc{ct}")
                nc.vector.tensor_sub(out=b_c, in0=gnb_sb[:, ct, :], in1=t2)
                # normalize on the (GN-phase-idle) ACT engine: h = a*x + b
                nc.scalar.activation(
                    out=h_sb[:, ct, :],
                    in_=x_t,
                    func=AF.Identity,
                    scale=a_c,
                    bias=b_c,
                )

            if _PHASES >= 2:
                # ---- phase 2: q, k, vT (fp8 DoubleRow over cc pairs) ----
                # k/v/q chunks interleaved so the PSUM->SBUF copies spread
                # evenly across ACT and DVE (v needs a per-free-dim bias add,
                # DVE-only; balance: ACT = 32k + 8q, DVE = 32v + 8q).
                def conv_mm(ps, w_or_h, rhs_is_w, oc_lo, n_lo, n_hi):
                    for cp in range(2):
                        if rhs_is_w:  # v conv: h stationary, weights moving
                            nc.tensor.matmul(
                                ps,
                                lhsT=h_sb[:, 2 * cp : 2 * cp + 2, n_lo:n_hi],
                                rhs=w_or_h[:, 2 * cp : 2 * cp + 2, :],
                                start=(cp == 0),
                                stop=(cp == 1),
                                perf_mode=DR,
                            )
                        else:
                            nc.tensor.matmul(
                                ps,
                                lhsT=w_or_h[:, 2 * cp : 2 * cp + 2, oc_lo : oc_lo + 128],
                                rhs=h_sb[:, 2 * cp : 2 * cp + 2, n_lo:n_hi],
                                start=(cp == 0),
                                stop=(cp == 1),
                                perf_mode=DR,
                            )

                qalt = 0
                for i in range(KT):
                    # k chunk i: oc = i // 8, nk = i % 8
                    oc, nk = i // 8, i % 8
                    ps = ps_a.tile([128, 512], F32, tag="s", bufs=3, name=f"kp{i}")
                    conv_mm(ps, wk_sb, False, oc * 128, nk * 512, (nk + 1) * 512)
                    nc.scalar.add(
                        out=k_sb[:, oc, nk * 512 : (nk + 1) * 512],
                        in_=ps,
                        add=kb_sb[:, oc, :],
                    )
                    # v chunk i
                    ps = ps_a.tile([128, 512], F32, tag="s", bufs=3, name=f"vp{i}")
                    conv_mm(ps, wv_sb, True, 0, i * 128, (i + 1) * 128)
                    nc.vector.tensor_add(out=v_sb[:, i, :], in0=ps, in1=vb_sb)
                    # q chunk every other i
                    if i % 2 == 0:
                        j = i // 2
                        oc, nq = j // 4, j % 4
                        ps = ps_a.tile([128, 512], F32, tag="s", bufs=3, name=f"qp{j}")
                        conv_mm(ps, wq_sb, False, oc * 128, nq * 512, (nq + 1) * 512)
                        dst = q_sb[:, oc, nq * 512 : (nq + 1) * 512]
                        if qalt % 2 == 0:
                            nc.scalar.add(out=dst, in_=ps, add=qb_sb[:, oc, :])
                        else:
                            nc.vector.tensor_scalar_add(
                                out=dst, in0=ps, scalar1=qb_sb[:, oc, :]
                            )
                        qalt += 1

            if _PHASES >= 3:
                # ---- phase 3: attention per 512-wide query chunk ----
                # Residual x for this core's query half, loaded as 4 wide
                # tiles (8KB/partition lines: ~4x better DMA efficiency than
                # per-(qc,oc) 2KB-line loads). y staged wide for the same
                # reason.
                xr_t = [
                    p_xr.tile([128, NQ], F32, tag="xr", name=f"xr{oc}")
                    for oc in range(CT)
                ]
                for oc in range(CT):
                    nc.sync.dma_start(
                        out=xr_t[oc], in_=x_d[oc * 128 : (oc + 1) * 128, 0:NQ]
                    )
                y_stage = [
                    p_y.tile([128, NQ], F32, tag="y", name=f"yst{oc}")
                    for oc in range(CT)
                ]
                def emit_proj(pqc, o_all, oc):
                    y_ps = ps_a.tile([128, 512], F32, tag="s", bufs=3, name=f"yp{pqc}_{oc}")
                    for dp in range(2):
                        nc.tensor.matmul(
                            y_ps,
                            lhsT=wp_sb[:, 2 * dp : 2 * dp + 2, oc * 128 : (oc + 1) * 128],
                            rhs=o_all[:, 2 * dp : 2 * dp + 2, :],
                            start=(dp == 0),
                            stop=(dp == 1),
                            perf_mode=DR,
                        )
                    # y = (y_ps + pb) + x  in one DVE op, into the wide stage
                    nc.vector.scalar_tensor_tensor(
                        out=y_stage[oc][:, pqc * 512 : (pqc + 1) * 512],
                        in0=y_ps,
                        scalar=pb_sb[:, oc, :],
                        in1=xr_t[oc][:, pqc * 512 : (pqc + 1) * 512],
                        op0=ALU.add,
                        op1=ALU.add,
                    )
                    if pqc == QC - 1:
                        nc.sync.dma_start(
                            out=y_d[oc * 128 : (oc + 1) * 128, :],
                            in_=y_stage[oc],
                        )

                prev_proj = None  # (qc, o_all) awaiting projection
                for qc in range(QC):
                    o_ps = [
                        ps_a.tile([128, 512], F32, tag="o", bufs=4, name=f"ops{qc}_{d}")
                        for d in range(CT)
                    ]
                    rs_ps = ps_a.tile([16, 512], F32, tag="rs", bufs=1, name=f"rs{qc}")
                    e_roll = p_e.tile([128, 6, 512], FP8, tag="e", name=f"e{qc}")

                    def emit_pair(j, o_ps=o_ps, rs_ps=rs_ps, e_roll=e_roll):
                        sl = (2 * j) % 6
                        e_pair = e_roll[:, sl : sl + 2, :]
                        nc.tensor.matmul(
                            rs_ps,
                            lhsT=ones_sb,
                            rhs=e_pair,
                            start=(j == 0),
                            stop=(j == KT // 2 - 1),
                            perf_mode=DR,
                        )
                        for d in range(CT):
                            nc.tensor.matmul(
                                o_ps[d],
                                lhsT=v_sb[:, 2 * j : 2 * j + 2, d * 128 : (d + 1) * 128],
                                rhs=e_pair,
                                start=(j == 0),
                                stop=(j == KT // 2 - 1),
                                perf_mode=DR,
                            )

                    pend = []
                    for kt in range(KT):
                        s_ps = ps_a.tile([128, 512], F32, tag="s", bufs=3, name=f"sp{qc}_{kt}")
                        for cp in range(2):
                            nc.tensor.matmul(
                                s_ps,
                                lhsT=k_sb[:, 2 * cp : 2 * cp + 2, kt * 128 : (kt + 1) * 128],
                                rhs=q_sb[:, 2 * cp : 2 * cp + 2, qc * 512 : (qc + 1) * 512],
                                start=(cp == 0),
                                stop=(cp == 1),
                                perf_mode=DR,
                            )
                        nc.scalar.activation(
                            out=e_roll[:, kt % 6, :],
                            in_=s_ps,
                            func=AF.Exp,
                            scale=SCALE,
                            bias=ebias_sb,
                        )
                        # previous chunk's projection, spread through this
                        # chunk's S-stream (keeps its softmax-normalize chain
                        # off the PE critical path)
                        if prev_proj is not None and kt in (2, 4, 6, 8):
                            emit_proj(prev_proj[0], prev_proj[1], (kt - 2) // 2)
                            if kt == 8:
                                prev_proj = None
                        if kt % 2 == 1:
                            # O-matmuls trail the S-matmuls by two kt-pairs so
                            # the PE never waits on the ACT exp.
                            pend.append(kt // 2)
                            if len(pend) > 2:
                                emit_pair(pend.pop(0))
                    for j in pend:
                        emit_pair(j)
                    rinv = p_rin.tile([1, 512], F32, tag="rin", name=f"rin{qc}")
                    nc.vector.reciprocal(out=rinv, in_=rs_ps[0:1, :])
                    rb_sb = p_rb.tile([128, 512], F32, tag="rb", name=f"rb{qc}")
                    nc.gpsimd.partition_broadcast(rb_sb[:, :], rinv[:, :])
                    o_all = p_ob.tile([128, CT, 512], FP8, tag="ob", name=f"ob{qc}")
                    for d in range(CT):
                        nc.vector.tensor_mul(out=o_all[:, d, :], in0=o_ps[d], in1=rb_sb)
                    prev_proj = (qc, o_all)
                # final chunk's projection
                for oc in range(CT):
                    emit_proj(QC - 1, prev_proj[1], oc)

    nc.finalize()
    _CACHE[key] = nc
    return nc


def _host_inputs(x, gn_w, gn_b, qw, qb, kw, kb, vw, vb, pw, pb):
    f8 = ml_dtypes.float8_e4m3
    f32 = np.float32
    bf = ml_dtypes.bfloat16
    xf = np.asarray(x, f32).reshape(B, C, N)

    def wt(w):
        return np.ascontiguousarray(np.asarray(w, f32).T).astype(f8)

    gmat = np.zeros((128, 8), f32)
    for p in range(128):
        gmat[p, p // GS] = 1.0 / GS  # average the 16 per-partition means
    gmat = gmat.astype(bf)  # 1/16 is exact in bf16
    hmat = np.zeros((8, 128), f32)
    for p in range(128):
        hmat[p // GS, p] = 1.0
    hmat = hmat.astype(bf)
    shared = {
        "qwT": wt(qw),
        "kwT": wt(kw),
        "vwT": wt(vw),
        "pwT": wt(pw),
        "qb": np.asarray(qb, f32).reshape(C, 1),
        "kb": np.asarray(kb, f32).reshape(C, 1),
        "vb": np.asarray(vb, f32).reshape(1, C),
        "pb": np.asarray(pb, f32).reshape(C, 1),
        "gnw": np.asarray(gn_w, f32).reshape(C, 1),
        "gnb": np.asarray(gn_b, f32).reshape(C, 1),
        "gmat": gmat,
        "hmat": hmat,
        "ones2": np.ones((128, 32), f8),
    }
    in_maps = []
    for core in range(8):
        s, half = core // 2, core % 2
        xs = np.ascontiguousarray(np.roll(xf[s], -NQ * half, axis=1))
        in_maps.append({"x": xs, **shared})
    return in_maps


def kernel(x, gn_w, gn_b, qw, qb, kw, kb, vw, vb, pw, pb):
    nc = _build_program()
    in_maps = _host_inputs(x, gn_w, gn_b, qw, qb, kw, kb, vw, vb, pw, pb)
    res = run_bass_kernel_spmd(nc, in_maps, list(range(8)))
    outs = res.results
    y = np.empty((B, C, N), np.float32)
    for s in range(B):
        y[s][:, :NQ] = outs[2 * s]["y"]
        y[s][:, NQ:] = outs[2 * s + 1]["y"]
    return y.reshape(B, C, 64, 64)
